# revision 6
# baseline (speedup 1.0000x reference)
"""Trainium2 Bass kernel for nn_CustomLoss_6330781795106.

Math (derived from the reference):
  p = softmax(y_pred, axis=1) clipped to [1e-7, 1]; th = 1/C
  per row i (label l_i, argmax a_i):
    py_i  = clip(exp(y[i,l_i]) / s_i, 1e-7, 1),  s_i = sum_j exp(y[i,j])
    nl_i  = (py_i - 1) * log(clip(1 - py_i, 1e-7, 1))
    ce2_i = a_i * log(py_i)
    mask_i = [second_largest(p_i) < th]
    pyD_i = mask_i * max(p_i)
  loss = sum(nl)/B + 0.01 * ( -prod(1 + pyD^2) * sum(ce2) )

Data-parallel over 8 cores (1024 rows each).  Per core, per 128-row tile:
  ACT   exp(f32 tile) -> bf16 tile, fused f32 row-sum accumulator (s).
  DVE   level-1: 32 chunk maxes of the bf16 exp tile (bf16 runs the DVE in
        its 2x/4x packed mode), top-8 chunk maxes + winning chunk indices.
        Top-TWO chunks are kept: bf16 rounding can tie two chunk maxes, and
        the f32-exact level-2 pass below disambiguates the winner.
  POOL  indirect-DMA gathers: the label logit (one op for all tiles) and the
        top-2 candidate chunks (2x128 raw f32 per row) per tile.
  DVE   level-2: top-8 + argmax position over the gathered 256 raw f32
        values -> exact row max, exact argmax index, (near-)exact 2nd max.
  A small batched epilogue turns the per-row scalars into per-partition
  partial sums; the host combines the 8 cores.

Raw Bass (no Tile): this walrus build encodes at most ONE sync-wait per
instruction, so cross-engine deps are standalone wait_ge ops with a static
semaphore schedule (op counters per engine, statically derived).
"""

import numpy as np

try:
    import concourse.bass as bass
except ImportError:  # pragma: no cover
    import sys

    sys.path.insert(0, "/opt/trn_rl_repo")
    import concourse.bass as bass

import concourse.mybir as mybir
from concourse.bass_utils import run_bass_kernel_spmd

B = 8192          # global batch
C = 4096          # classes
NCORES = 8
R = B // NCORES   # rows per core (1024)
P = 128           # partitions
T = R // P        # row-tiles per core (8)
NCH = C // 128    # 128-wide chunks per row (32)
TH = 1.0 / C
F32 = mybir.dt.float32
BF16 = mybir.dt.bfloat16
U32 = mybir.dt.uint32
I32 = mybir.dt.int32
AF = mybir.ActivationFunctionType
ALU = mybir.AluOpType
X = mybir.AxisListType.X


def _build(R=R):
    T = R // P
    nc = bass.Bass("TRN2", debug=False)
    y_d = nc.dram_tensor("y", [R, C], F32, kind="ExternalInput")
    off_d = nc.dram_tensor("off", [P, T], U32, kind="ExternalInput")
    rb2_d = nc.dram_tensor("rb2", [P, 2], F32, kind="ExternalInput")
    out_d = nc.dram_tensor("out", [P, 4], F32, kind="ExternalOutput")

    from contextlib import ExitStack
    with ExitStack() as ctx:
        def sb(name, shape, dt=F32):
            return ctx.enter_context(nc.sbuf_tensor(name, shape, dt))

        yt = sb("yt", [P, T * C])            # raw shard: 128 KiB/partition
        et = sb("et", [P, T * C], BF16)      # exp'd shard (bf16): 64 KiB
        s8 = sb("s8", [P, T]); ylab = sb("ylab", [P, T])
        offs = sb("offs", [P, T], U32)
        rb2p = sb("rb2p", [P, 2])
        rt = sb("rt", [P, NCH * T], BF16)    # level-1 chunk maxes
        cm8 = sb("cm8", [P, 8 * T], BF16)    # top-8 chunk maxes per tile
        ci8 = sb("ci8", [P, 8 * T], U32)     # their chunk indices
        cif2 = sb("cif2", [P, 2 * T])        # top-2 chunk idx as f32
        offwf = sb("offwf", [P, 2 * T])      # gather offsets (f32)
        offw = sb("offw", [P, 2 * T], I32)   # gather offsets (int)
        wraw = sb("wraw", [P, 256 * T])      # gathered top-2 chunks (f32)
        cw8 = sb("cw8", [P, 8 * T])          # top-8 of the 256 raw values
        wq8 = sb("wq8", [P, 8 * T], U32)     # their positions in [0,256)
        warm = sb("warm", [P, 1])            # ACT table preload dummy
        # epilogue scratch, [P, T] each
        rs = sb("rs", [P, T]); u = sb("u", [P, T]); pyr = sb("pyr", [P, T])
        py = sb("py", [P, T]); t1 = sb("t1", [P, T]); t1c = sb("t1c", [P, T])
        l1 = sb("l1", [P, T]); lp = sb("lp", [P, T]); nlp = sb("nlp", [P, T])
        ce2 = sb("ce2", [P, T]); qf = sb("qf", [P, T]); c1f = sb("c1f", [P, T])
        c2f = sb("c2f", [P, T]); cnd = sb("cnd", [P, T]); dsl = sb("dsl", [P, T])
        m1 = sb("m1", [P, T]); csl = sb("csl", [P, T]); kk = sb("kk", [P, T])
        a8 = sb("a8", [P, T]); me = sb("me", [P, T]); ve = sb("ve", [P, T])
        mp = sb("mp", [P, T]); q2 = sb("q2", [P, T]); msk = sb("msk", [P, T])
        pyD = sb("pyD", [P, T]); sq = sb("sq", [P, T]); lw = sb("lw", [P, T])
        outsb = sb("outsb", [P, 4])

        # strided [P, T] views over the per-tile top-8 outputs
        mxv = cw8[:].rearrange("p (t e) -> p t e", e=8)[:, :, 0]   # row max (f32)
        v2v = cw8[:].rearrange("p (t e) -> p t e", e=8)[:, :, 1]   # 2nd max (f32)
        q0v = wq8[:].rearrange("p (t e) -> p t e", e=8)[:, :, 0]   # pos in [0,256)
        c0v = ci8[:].rearrange("p (t e) -> p t e", e=8)[:, :, 0]   # chunk 1
        c1v = ci8[:].rearrange("p (t e) -> p t e", e=8)[:, :, 1]   # chunk 2

        sem_y = ctx.enter_context(nc.semaphore("sem_y"))      # out store
        sem_t = [ctx.enter_context(nc.semaphore(f"sem_t{t}")) for t in range(T)]
        sem_sw = ctx.enter_context(nc.semaphore("sem_sw"))    # offs+rb2 loads
        sem_lab = ctx.enter_context(nc.semaphore("sem_lab"))  # label gather
        # one sem per winner gather: gathers on the dynamic queue may be
        # modeled as completing out of order, so a shared counting sem
        # cannot tell WHICH gather has landed
        sem_g = [ctx.enter_context(nc.semaphore(f"sem_g{t}")) for t in range(T)]
        sem_act = ctx.enter_context(nc.semaphore("sem_act"))  # ACT op counter
        sem_dve = ctx.enter_context(nc.semaphore("sem_dve"))  # DVE op counter
        sem_gv = ctx.enter_context(nc.semaphore("sem_gv"))    # gather offs ready
        sem_w = ctx.enter_context(nc.semaphore("sem_w"))      # warm memset
        block = ctx.enter_context(nc.Block())

        # ---- static DVE op counts (sem_dve values) ----
        # main loop: 5 counted ops per tile (reduce, max8, max_index, cast,
        # sts); the offw cast increments sem_gv instead.  lvl2(j) (2 ops) is
        # placed after the tile-(j+2) group; lvl2(6), lvl2(7) follow the loop.
        n = 0
        n_red = [0] * T          # count after tile t's level-1 reduce
        for t in range(T):
            n_red[t] = n + 1
            n += 5
            if t >= 2:
                n += 2           # lvl2(t-2)
        n += 4                   # lvl2(6), lvl2(7)
        N_LVL2 = n               # all level-2 done
        N_T1C = n + 5            # rs, pyr, py, t1, t1c
        N_A8 = N_T1C + 9         # qf..a8
        N_SQ = N_A8 + 5          # mp, q2, msk, pyD, sq
        N_FINAL = N_SQ + 6       # nlp, ce2, 4 reduces
        # ---- ACT op counts (sem_act values) ----
        A_EXP = lambda t: 2 + t  # warm is op 1
        A_U = 2 + T              # 10
        A_VE = A_U + 2           # me, ve
        A_LP = A_VE + 2          # l1, lp
        A_LW = A_LP + 1

        @block.sync
        def _(sp):
            for t in range(T):
                sp.dma_start(yt[:, t * C:(t + 1) * C],
                             y_d[t * P:(t + 1) * P, :]).then_inc(sem_t[t], 16)
            sp.wait_ge(sem_dve, N_FINAL)
            sp.dma_start(out_d[:, :], outsb[:]).then_inc(sem_y, 16)
            sp.wait_ge(sem_y, 16)

        @block.gpsimd
        def _(pl):
            pl.memset(warm[:], 0.0).then_inc(sem_w, 1)
            pl.dma_start(offs[:], off_d[:, :]).then_inc(sem_sw, 16)
            pl.dma_start(rb2p[:], rb2_d[:, :]).then_inc(sem_sw, 16)
            pl.wait_ge(sem_sw, 32)
            # label logits: flat element indices into [R, C]
            pl.indirect_dma_start(
                out=ylab[:], out_offset=None,
                in_=y_d[:, :],
                in_offset=bass.IndirectOffsetOnAxis(ap=offs[:], axis=1),
            ).then_inc(sem_lab, 16)
            # top-2 candidate chunks per row: 2 x 128 contiguous f32
            for t in range(T):
                pl.wait_ge(sem_gv, t + 1)
                pl.indirect_dma_start(
                    out=wraw[:, t * 256:(t + 1) * 256], out_offset=None,
                    in_=y_d[:, :],
                    in_offset=bass.IndirectOffsetOnAxis(
                        ap=offw[:, 2 * t:2 * t + 2], axis=1),
                    element_offset=t * P * C,
                ).then_inc(sem_g[t], 16)

        @block.scalar
        def _(act):
            # dummy exp: pulls the ACT function-table load off the critical
            # path (it otherwise serializes with the first tile's exp)
            act.wait_ge(sem_w, 1)
            act.activation(out=warm[:], in_=warm[:],
                           func=AF.Exp).then_inc(sem_act, 1)
            # main pass: exp to bf16 + fused f32 row-sum; no max-subtraction
            # (logits are standard normals, exp() is safe in f32)
            for t in range(T):
                act.wait_ge(sem_t[t], 16)
                act.activation(out=et[:, t * C:(t + 1) * C],
                               in_=yt[:, t * C:(t + 1) * C], func=AF.Exp,
                               accum_out=s8[:, t:t + 1]).then_inc(sem_act, 1)
            act.wait_ge(sem_lab, 16)
            act.activation(out=u[:], in_=ylab[:],
                           func=AF.Exp).then_inc(sem_act, 1)        # A_U
            act.wait_ge(sem_dve, N_LVL2)
            act.activation(out=me[:], in_=mxv,
                           func=AF.Exp).then_inc(sem_act, 1)
            act.activation(out=ve[:], in_=v2v,
                           func=AF.Exp).then_inc(sem_act, 1)        # A_VE
            act.wait_ge(sem_dve, N_T1C)
            act.activation(out=l1[:], in_=t1c[:],
                           func=AF.Ln).then_inc(sem_act, 1)
            act.activation(out=lp[:], in_=py[:],
                           func=AF.Ln).then_inc(sem_act, 1)         # A_LP
            act.wait_ge(sem_dve, N_SQ)
            act.activation(out=lw[:], in_=sq[:], func=AF.Ln,
                           bias=1.0).then_inc(sem_act, 1)           # A_LW

        @block.vector
        def _(dve):
            # DVE same-engine RAW deps need an explicit self-semaphore
            # (then_inc + wait) between dependent ops.
            state = {"n": 0}

            def step(inst):
                inst.then_inc(sem_dve, 1)
                state["n"] += 1

            def dwait():
                dve.wait_ge(sem_dve, state["n"])

            def lvl2(j):
                dve.wait_ge(sem_g[j], 16)
                wsl = wraw[:, j * 256:(j + 1) * 256]
                sl = slice(8 * j, 8 * (j + 1))
                step(dve.max(out=cw8[:, sl], in_=wsl))
                dwait()
                step(dve.max_index(out=wq8[:, sl], in_max=cw8[:, sl],
                                   in_values=wsl))

            dve.wait_ge(sem_sw, 32)                 # rb2p loaded
            for t in range(T):
                dve.wait_ge(sem_act, A_EXP(t))
                et3 = et[:, t * C:(t + 1) * C].rearrange(
                    "p (a c) -> p a c", c=128)
                rsl = rt[:, NCH * t:NCH * (t + 1)]
                sl = slice(8 * t, 8 * (t + 1))
                assert state["n"] + 1 == n_red[t]
                # level 1: 32 chunk maxes (bf16 -> packed 2x/4x DVE mode)
                step(dve.tensor_reduce(rsl, et3, axis=X, op=ALU.max))
                dwait()
                step(dve.max(out=cm8[:, sl], in_=rsl))
                dwait()
                step(dve.max_index(out=ci8[:, sl], in_max=cm8[:, sl],
                                   in_values=rsl))
                dwait()
                step(dve.tensor_copy(cif2[:, 2 * t:2 * t + 2],
                                     ci8[:, 8 * t:8 * t + 2].bitcast(I32)))
                dwait()
                step(dve.scalar_tensor_tensor(
                    out=offwf[:, 2 * t:2 * t + 2],
                    in0=cif2[:, 2 * t:2 * t + 2], scalar=128.0, in1=rb2p[:],
                    op0=ALU.mult, op1=ALU.add))
                dwait()
                # offw consumed by POOL only: sole sem update goes to sem_gv
                dve.tensor_copy(offw[:, 2 * t:2 * t + 2],
                                offwf[:, 2 * t:2 * t + 2]).then_inc(sem_gv, 1)
                if t >= 2:
                    lvl2(t - 2)
            lvl2(T - 2)
            lvl2(T - 1)
            assert state["n"] == N_LVL2

            # ---- epilogue: batched [P, T] ops ----
            dve.wait_ge(sem_act, A_EXP(T - 1))      # s8 complete
            step(dve.reciprocal(rs[:], s8[:]))
            dve.wait_ge(sem_act, A_U)
            dwait()
            step(dve.tensor_mul(pyr[:], u[:], rs[:]))
            dwait()
            step(dve.tensor_scalar(py[:], pyr[:], 1e-7, 1.0, op0=ALU.max,
                                   op1=ALU.min))
            dwait()
            step(dve.tensor_scalar(t1[:], py[:], -1.0, 1.0, op0=ALU.mult,
                                   op1=ALU.add))    # 1 - py
            dwait()
            step(dve.tensor_scalar_max(t1c[:], t1[:], 1e-7))
            assert state["n"] == N_T1C
            # argmax index assembly: a = 128*(q<128 ? c1 : c2) + q mod 128
            step(dve.tensor_copy(qf[:], q0v.bitcast(I32)))
            step(dve.tensor_copy(c1f[:], c0v.bitcast(I32)))
            step(dve.tensor_copy(c2f[:], c1v.bitcast(I32)))
            dwait()
            step(dve.tensor_scalar(cnd[:], qf[:], 128.0, None, op0=ALU.is_ge))
            step(dve.tensor_tensor(out=dsl[:], in0=c2f[:], in1=c1f[:],
                                   op=ALU.subtract))
            dwait()
            step(dve.tensor_mul(m1[:], cnd[:], dsl[:]))
            dwait()
            step(dve.tensor_add(csl[:], c1f[:], m1[:]))
            dwait()
            step(dve.tensor_tensor(out=kk[:], in0=csl[:], in1=cnd[:],
                                   op=ALU.subtract))
            dwait()
            step(dve.scalar_tensor_tensor(out=a8[:], in0=kk[:], scalar=128.0,
                                          in1=qf[:], op0=ALU.mult,
                                          op1=ALU.add))
            assert state["n"] == N_A8
            dve.wait_ge(sem_act, A_VE)
            step(dve.tensor_mul(mp[:], me[:], rs[:]))   # max prob
            step(dve.tensor_mul(q2[:], ve[:], rs[:]))   # 2nd prob
            dwait()
            step(dve.tensor_scalar(msk[:], q2[:], TH, None, op0=ALU.is_lt))
            dwait()
            step(dve.tensor_mul(pyD[:], msk[:], mp[:]))
            dwait()
            step(dve.tensor_mul(sq[:], pyD[:], pyD[:]))
            assert state["n"] == N_SQ
            dve.wait_ge(sem_act, A_LP)
            step(dve.tensor_mul(nlp[:], t1[:], l1[:]))  # host negates
            step(dve.tensor_mul(ce2[:], a8[:], lp[:]))
            dwait()
            step(dve.tensor_reduce(outsb[:, 0:1], nlp[:], axis=X, op=ALU.add))
            step(dve.tensor_reduce(outsb[:, 1:2], ce2[:], axis=X, op=ALU.add))
            dve.wait_ge(sem_act, A_LW)
            step(dve.tensor_reduce(outsb[:, 2:3], lw[:], axis=X, op=ALU.add))
            dwait()
            step(dve.tensor_reduce(outsb[:, 3:4], a8[:], axis=X, op=ALU.add))
            assert state["n"] == N_FINAL, state["n"]
    return nc


def _in_maps(y, lab):
    maps = []
    for c in range(NCORES):
        ys = np.ascontiguousarray(y[c * R:(c + 1) * R])
        labs = lab[c * R:(c + 1) * R].astype(np.int64)
        r = np.arange(R, dtype=np.int64)
        flat = (r * C + labs).astype(np.uint32)
        off = np.ascontiguousarray(flat.reshape(T, P).T)  # [P, T]
        rb2 = np.repeat((np.arange(P, dtype=np.float32) * C)[:, None], 2,
                        axis=1)
        maps.append({"y": ys, "off": off, "rb2": rb2})
    return maps


def _combine(results):
    nlp_sum = 0.0
    ce2_sum = 0.0
    lw_sum = 0.0
    for c in range(NCORES):
        o = np.asarray(results[c]["out"], dtype=np.float64)
        nlp_sum += o[:, 0].sum()
        ce2_sum += o[:, 1].sum()
        lw_sum += o[:, 2].sum()
    nl = -nlp_sum / float(B)
    pl = -np.exp(lw_sum) * ce2_sum
    return np.array([nl + 0.01 * pl], dtype=np.float32)


def kernel(y_pred, y_true2):
    y = np.ascontiguousarray(np.asarray(y_pred, dtype=np.float32))
    lab = np.asarray(y_true2).astype(np.int64)
    assert y.shape == (B, C) and lab.shape == (B,)
    nc = _build()
    res = run_bass_kernel_spmd(nc, _in_maps(y, lab),
                               core_ids=list(range(NCORES))).results
    return _combine(res)


# revision 14
# speedup vs baseline: 1.1339x; 1.1339x over previous
"""Trainium2 Bass kernel for nn_CustomLoss_6330781795106.

Math (derived from the reference):
  p = softmax(y_pred, axis=1) clipped to [1e-7, 1]; th = 1/C
  per row i (label l_i, argmax a_i):
    py_i  = clip(exp(y[i,l_i]) / s_i, 1e-7, 1),  s_i = sum_j exp(y[i,j])
    nl_i  = (py_i - 1) * log(clip(1 - py_i, 1e-7, 1))
    ce2_i = a_i * log(py_i)
    mask_i = [second_largest(p_i) < th]
    pyD_i = mask_i * max(p_i)
  loss = sum(nl)/B + 0.01 * ( -prod(1 + pyD^2) * sum(ce2) )

Data-parallel over 8 cores (1024 rows each).  All max/argmax work runs in
the raw-y domain (exp is monotone), decoupling it from the exp pass:

  ACT   exp(f32 tile) with fused f32 row-sum accumulator; the exp output
        itself is dead (ping-pong scratch) -- only the sum is used.  ACT
        also computes the winner-chunk gather offsets (Relu with scale=128
        and per-partition bias) in its idle slots, keeping them off the
        DVE critical path.
  DVE   level-1: 32 chunk maxes per tile (tensor_reduce, the irreducible
        1-elem/cycle scan), then top-8 chunk maxes + winning chunk index.
        Level-2(j) is interleaved right after level-1(t) so back-to-back
        RAW latencies are hidden.
  POOL  indirect-DMA gathers: label logits and the winner chunk (128 raw
        f32 per row) per tile.
  DVE   level-2: top-8 + argmax position over the winner chunk -> exact
        row max, exact argmax index, 2nd-in-chunk (for the exact 2nd max).
  A small batched epilogue turns per-row scalars into per-partition partial
  sums; the host combines the 8 cores.

Raw Bass (no Tile): this walrus build encodes at most ONE sync-wait per
instruction, so cross-engine deps are standalone wait_ge ops against
per-engine op counters (values recorded while the streams are built).
"""

import numpy as np

try:
    import concourse.bass as bass
except ImportError:  # pragma: no cover
    import sys

    sys.path.insert(0, "/opt/trn_rl_repo")
    import concourse.bass as bass

import concourse.mybir as mybir
from concourse.bass_utils import run_bass_kernel_spmd

B = 8192          # global batch
C = 4096          # classes
NCORES = 8
R = B // NCORES   # rows per core (1024)
P = 128           # partitions
T = R // P        # row-tiles per core (8)
NCH = C // 128    # 128-wide chunks per row (32)
TH = 1.0 / C
F32 = mybir.dt.float32
U32 = mybir.dt.uint32
I32 = mybir.dt.int32
AF = mybir.ActivationFunctionType
ALU = mybir.AluOpType
X = mybir.AxisListType.X


def _build(R=R):
    T = R // P
    nc = bass.Bass("TRN2", debug=False)
    y_d = nc.dram_tensor("y", [R, C], F32, kind="ExternalInput")
    off_d = nc.dram_tensor("off", [P, T], U32, kind="ExternalInput")
    rb_d = nc.dram_tensor("rb", [P, 1], F32, kind="ExternalInput")
    out_d = nc.dram_tensor("out", [P, 4], F32, kind="ExternalOutput")

    from contextlib import ExitStack
    with ExitStack() as ctx:
        def sb(name, shape, dt=F32):
            return ctx.enter_context(nc.sbuf_tensor(name, shape, dt))

        yt = sb("yt", [P, T * C])            # raw shard: 128 KiB/partition
        ex = sb("ex", [P, 2 * C])            # dead exp output (accum only)
        s8 = sb("s8", [P, T]); ylab = sb("ylab", [P, T])
        offs = sb("offs", [P, T], U32)
        rbp = sb("rbp", [P, 1])
        rt = sb("rt", [P, NCH * T])          # level-1 chunk maxes (f32)
        cm8 = sb("cm8", [P, 8 * T])          # top-8 chunk maxes per tile
        ci8 = sb("ci8", [P, 8 * T], U32)     # their chunk indices
        offw = sb("offw", [P, T], I32)       # winner gather offsets
        wraw = sb("wraw", [P, 128 * T])      # gathered winner chunks (f32)
        cw8 = sb("cw8", [P, 8 * T])          # top-8 of each winner chunk
        wq8 = sb("wq8", [P, 8 * T], U32)     # their positions in [0,128)
        warm = sb("warm", [P, 1])            # ACT table preload dummy
        # epilogue scratch, [P, T] each
        rs = sb("rs", [P, T]); u = sb("u", [P, T]); pyr = sb("pyr", [P, T])
        py = sb("py", [P, T]); t1 = sb("t1", [P, T]); t1c = sb("t1c", [P, T])
        l1 = sb("l1", [P, T]); lp = sb("lp", [P, T]); nlp = sb("nlp", [P, T])
        ce2 = sb("ce2", [P, T]); qf = sb("qf", [P, T]); c1f = sb("c1f", [P, T])
        a8 = sb("a8", [P, T]); v8t = sb("v8t", [P, T])
        me = sb("me", [P, T]); ve = sb("ve", [P, T])
        mp = sb("mp", [P, T]); q2 = sb("q2", [P, T]); msk = sb("msk", [P, T])
        pyD = sb("pyD", [P, T]); sq = sb("sq", [P, T]); lw = sb("lw", [P, T])
        outsb = sb("outsb", [P, 4])

        # strided [P, T] views over the per-tile top-8 outputs
        mxv = cw8[:].rearrange("p (t e) -> p t e", e=8)[:, :, 0]   # row max
        w2v = cw8[:].rearrange("p (t e) -> p t e", e=8)[:, :, 1]   # 2nd in chunk
        c2v = cm8[:].rearrange("p (t e) -> p t e", e=8)[:, :, 1]   # 2nd chunk max
        q0v = wq8[:].rearrange("p (t e) -> p t e", e=8)[:, :, 0]   # pos in chunk
        c0v = ci8[:].rearrange("p (t e) -> p t e", e=8)[:, :, 0]   # chunk idx

        sem_y = ctx.enter_context(nc.semaphore("sem_y"))      # out store
        sem_t = [ctx.enter_context(nc.semaphore(f"sem_t{t}")) for t in range(T)]
        sem_sw = ctx.enter_context(nc.semaphore("sem_sw"))    # offs+rb loads
        sem_lab = ctx.enter_context(nc.semaphore("sem_lab"))  # label gather
        # one sem per winner gather: gathers on the dynamic queue may
        # complete out of order, so a shared counting sem cannot tell WHICH
        # gather has landed
        sem_g = [ctx.enter_context(nc.semaphore(f"sem_g{t}")) for t in range(T)]
        sem_act = ctx.enter_context(nc.semaphore("sem_act"))  # ACT op counter
        sem_dve = ctx.enter_context(nc.semaphore("sem_dve"))  # DVE op counter
        sem_gv = ctx.enter_context(nc.semaphore("sem_gv"))    # gather offs ready
        sem_w = ctx.enter_context(nc.semaphore("sem_w"))      # warm memset
        block = ctx.enter_context(nc.Block())

        # ACT op counts on sem_act (offw ops increment sem_gv instead)
        A_EXP = lambda t: 2 + t  # warm is op 1
        A_U = 2 + T              # 10
        A_LP = A_U + 2           # l1, lp
        A_VE = A_LP + 2          # me, ve
        A_LW = A_VE + 1
        # DVE counts recorded while the stream is built (read by ACT/SP)
        DN = {}

        def y3(t):
            return yt[:, t * C:(t + 1) * C].rearrange("p (a c) -> p a c", c=128)

        @block.vector
        def _(dve):
            # DVE same-engine RAW deps need an explicit self-semaphore
            # (then_inc + wait) between dependent ops.
            state = {"n": 0}

            def step(inst):
                inst.then_inc(sem_dve, 1)
                state["n"] += 1

            def dwait():
                dve.wait_ge(sem_dve, state["n"])

            def lvl2(j):
                dve.wait_ge(sem_g[j], 16)
                wsl = wraw[:, j * 128:(j + 1) * 128]
                sl = slice(8 * j, 8 * (j + 1))
                step(dve.max(out=cw8[:, sl], in_=wsl))
                dwait()
                step(dve.max_index(out=wq8[:, sl], in_max=cw8[:, sl],
                                   in_values=wsl))

            for t in range(T):
                dve.wait_ge(sem_t[t], 16)
                # level 1: 32 chunk maxes -- the irreducible full scan
                step(dve.tensor_reduce(rt[:, NCH * t:NCH * (t + 1)], y3(t),
                                       axis=X, op=ALU.max))
                if t >= 2:
                    lvl2(t - 2)   # hides the reduce->max8 RAW latency
                dwait()
                rsl = rt[:, NCH * t:NCH * (t + 1)]
                sl = slice(8 * t, 8 * (t + 1))
                step(dve.max(out=cm8[:, sl], in_=rsl))
                dwait()
                step(dve.max_index(out=ci8[:, sl], in_max=cm8[:, sl],
                                   in_values=rsl))
                DN[f"mi{t}"] = state["n"]
            lvl2(T - 2)
            lvl2(T - 1)

            # ---- epilogue: batched [P, T] ops ----
            dve.wait_ge(sem_act, A_EXP(T - 1))      # s8 complete
            step(dve.reciprocal(rs[:], s8[:]))
            dve.wait_ge(sem_act, A_U)
            dwait()
            step(dve.tensor_mul(pyr[:], u[:], rs[:]))
            dwait()
            step(dve.tensor_scalar(py[:], pyr[:], 1e-7, 1.0, op0=ALU.max,
                                   op1=ALU.min))
            dwait()
            step(dve.tensor_scalar(t1[:], py[:], -1.0, 1.0, op0=ALU.mult,
                                   op1=ALU.add))    # 1 - py
            dwait()
            step(dve.tensor_scalar_max(t1c[:], t1[:], 1e-7))
            DN["t1c"] = state["n"]
            # argmax index: a = 128 * chunk + pos-in-chunk
            step(dve.tensor_copy(qf[:], q0v.bitcast(I32)))
            step(dve.tensor_copy(c1f[:], c0v.bitcast(I32)))
            dwait()
            step(dve.scalar_tensor_tensor(out=a8[:], in0=c1f[:], scalar=128.0,
                                          in1=qf[:], op0=ALU.mult,
                                          op1=ALU.add))
            # exact 2nd max: max(2nd chunk max, 2nd within winner chunk)
            step(dve.tensor_tensor(out=v8t[:], in0=c2v, in1=w2v, op=ALU.max))
            DN["v8t"] = state["n"]
            dve.wait_ge(sem_act, A_VE)
            step(dve.tensor_mul(mp[:], me[:], rs[:]))   # max prob
            step(dve.tensor_mul(q2[:], ve[:], rs[:]))   # 2nd prob
            dwait()
            step(dve.tensor_scalar(msk[:], q2[:], TH, None, op0=ALU.is_lt))
            dwait()
            step(dve.tensor_mul(pyD[:], msk[:], mp[:]))
            dwait()
            step(dve.tensor_mul(sq[:], pyD[:], pyD[:]))
            DN["sq"] = state["n"]
            dve.wait_ge(sem_act, A_LP)
            step(dve.tensor_mul(nlp[:], t1[:], l1[:]))  # host negates
            step(dve.tensor_mul(ce2[:], a8[:], lp[:]))
            dwait()
            step(dve.tensor_reduce(outsb[:, 0:1], nlp[:], axis=X, op=ALU.add))
            step(dve.tensor_reduce(outsb[:, 1:2], ce2[:], axis=X, op=ALU.add))
            dve.wait_ge(sem_act, A_LW)
            step(dve.tensor_reduce(outsb[:, 2:3], lw[:], axis=X, op=ALU.add))
            dwait()
            step(dve.tensor_reduce(outsb[:, 3:4], a8[:], axis=X, op=ALU.add))
            DN["final"] = state["n"]

        @block.gpsimd
        def _(pl):
            pl.memset(warm[:], 0.0).then_inc(sem_w, 1)
            pl.dma_start(offs[:], off_d[:, :]).then_inc(sem_sw, 16)
            pl.dma_start(rbp[:], rb_d[:, :]).then_inc(sem_sw, 16)
            pl.wait_ge(sem_sw, 32)
            # label logits: flat element indices into [R, C]
            pl.indirect_dma_start(
                out=ylab[:], out_offset=None,
                in_=y_d[:, :],
                in_offset=bass.IndirectOffsetOnAxis(ap=offs[:], axis=1),
            ).then_inc(sem_lab, 16)
            # winner chunks: 128 contiguous raw f32 per row
            for j in range(T):
                pl.wait_ge(sem_gv, j + 1)
                pl.indirect_dma_start(
                    out=wraw[:, j * 128:(j + 1) * 128], out_offset=None,
                    in_=y_d[:, :],
                    in_offset=bass.IndirectOffsetOnAxis(
                        ap=offw[:, j:j + 1], axis=1),
                    element_offset=j * P * C,
                ).then_inc(sem_g[j], 16)

        @block.scalar
        def _(act):
            # dummy exp: pulls the ACT function-table load off the critical
            # path (it otherwise serializes with the first tile's exp)
            act.wait_ge(sem_w, 1)
            act.activation(out=warm[:], in_=warm[:],
                           func=AF.Exp).then_inc(sem_act, 1)
            act.wait_ge(sem_sw, 32)                 # rbp (offw bias) loaded

            def offw_op(t):
                # offw[t] = 128 * ci8[t][0] + p*C, computed away from the
                # DVE: Relu(in*128 + rb) with all-positive inputs is exact
                act.wait_ge(sem_dve, DN[f"mi{t}"])
                act.activation(out=offw[:, t:t + 1],
                               in_=ci8[:, 8 * t:8 * t + 1], func=AF.Relu,
                               scale=128.0, bias=rbp[:]).then_inc(sem_gv, 1)

            # main pass: only the fused f32 row-sum accumulator is consumed;
            # no max-subtraction (logits are standard normals, exp() is safe)
            for t in range(T):
                act.wait_ge(sem_t[t], 16)
                if t >= 2:
                    # ping-pong WAW: wait until the exp two tiles back retired
                    act.wait_ge(sem_act, A_EXP(t - 2))
                eslot = ex[:, (t % 2) * C:(t % 2 + 1) * C]
                act.activation(out=eslot, in_=yt[:, t * C:(t + 1) * C],
                               func=AF.Exp,
                               accum_out=s8[:, t:t + 1]).then_inc(sem_act, 1)
                if t >= 1:
                    offw_op(t - 1)
            offw_op(T - 1)
            act.wait_ge(sem_lab, 16)
            act.activation(out=u[:], in_=ylab[:],
                           func=AF.Exp).then_inc(sem_act, 1)        # A_U
            act.wait_ge(sem_dve, DN["t1c"])
            act.activation(out=l1[:], in_=t1c[:],
                           func=AF.Ln).then_inc(sem_act, 1)
            act.activation(out=lp[:], in_=py[:],
                           func=AF.Ln).then_inc(sem_act, 1)         # A_LP
            act.wait_ge(sem_dve, DN["v8t"])
            act.activation(out=me[:], in_=mxv,
                           func=AF.Exp).then_inc(sem_act, 1)
            act.activation(out=ve[:], in_=v8t[:],
                           func=AF.Exp).then_inc(sem_act, 1)        # A_VE
            act.wait_ge(sem_dve, DN["sq"])
            act.activation(out=lw[:], in_=sq[:], func=AF.Ln,
                           bias=1.0).then_inc(sem_act, 1)           # A_LW

        @block.sync
        def _(sp):
            for t in range(T):
                sp.dma_start(yt[:, t * C:(t + 1) * C],
                             y_d[t * P:(t + 1) * P, :]).then_inc(sem_t[t], 16)
            sp.wait_ge(sem_dve, DN["final"])
            sp.dma_start(out_d[:, :], outsb[:]).then_inc(sem_y, 16)
            sp.wait_ge(sem_y, 16)
    return nc


def _in_maps(y, lab):
    maps = []
    for c in range(NCORES):
        ys = np.ascontiguousarray(y[c * R:(c + 1) * R])
        labs = lab[c * R:(c + 1) * R].astype(np.int64)
        r = np.arange(R, dtype=np.int64)
        flat = (r * C + labs).astype(np.uint32)
        off = np.ascontiguousarray(flat.reshape(T, P).T)  # [P, T]
        rb = (np.arange(P, dtype=np.float32) * C).reshape(P, 1)
        maps.append({"y": ys, "off": off, "rb": rb})
    return maps


def _combine(results):
    nlp_sum = 0.0
    ce2_sum = 0.0
    lw_sum = 0.0
    for c in range(NCORES):
        o = np.asarray(results[c]["out"], dtype=np.float64)
        nlp_sum += o[:, 0].sum()
        ce2_sum += o[:, 1].sum()
        lw_sum += o[:, 2].sum()
    nl = -nlp_sum / float(B)
    pl = -np.exp(lw_sum) * ce2_sum
    return np.array([nl + 0.01 * pl], dtype=np.float32)


def kernel(y_pred, y_true2):
    y = np.ascontiguousarray(np.asarray(y_pred, dtype=np.float32))
    lab = np.asarray(y_true2).astype(np.int64)
    assert y.shape == (B, C) and lab.shape == (B,)
    nc = _build()
    res = run_bass_kernel_spmd(nc, _in_maps(y, lab),
                               core_ids=list(range(NCORES))).results
    return _combine(res)
